# revision 18
# baseline (speedup 1.0000x reference)
"""LipschitzRNN Trainium2 kernel.

Math (per reference):
    bA = 0.5*exp(-bA_z^2)+0.5 ; bW likewise
    A = (1-bA)(MA+MA.T) + bA(MA-MA.T) - YA*I
    C = (1-bA)(MW+MW.T) + bW(MW-MW.T) - YW*I
    X_{t+1} = X_t + STEP*(A@X_t + tanh(C@X_t + by))   (column-state X: [n, bs])
    out[b, t, :] = X_t[:, b]

Device strategy (8-way batch data-parallel, b=32/core, no collectives):
  - State kept as [n(partitions), b] in SBUF: two k-chunks side by side
    [128, 64], so elementwise ops use all 128 partitions, the per-partition
    bias `by` fuses into ScalarE's tanh, and no per-step transpose is needed
    for the recurrence itself.
  - Matmul operands in fp16 (FWL-eligible weight loads, single-pass PE).
    A fp32 "master" state is kept alongside the fp16 copy, with the
    UNFOLDED update  X' = X + STEP*(A@xq + tanh(C@xq + by)) , so fp16
    quantization of weights/state only enters through STEP-scaled paths
    (measured end-to-end relative error ~9e-4 over 511 steps).
  - Per step: 8 matmuls (V=C@xq first -> feeds tanh chain; U=A@xq), then
    P = X + STEP*U on VectorE while ScalarE runs the two biased tanh chunks,
    then xq' (fp16) and X' (fp32) = (tanh*STEP)+P.
  - Output rows need [b, n] layout: PE-transposes of xq (emitted one
    iteration late so next-step matmuls win scheduler priority), batched
    4 steps per SBUF stage copy (split between VectorE and ScalarE) and
    per 128KB DMA.
"""

import os
import numpy as np

N = 256
BS = 256
TMAX = 512
STEP = 0.01
YA = 0.001
YW = 0.001
NCORES = 8
BLOC = BS // NCORES  # 32

LAST_RESULT = None  # BassKernelResults of the most recent run (for test harness)


def _build(n_steps):
    from concourse import bacc, tile
    import concourse.mybir as mybir
    from concourse.masks import make_identity

    F32 = mybir.dt.float32
    F16 = mybir.dt.float16
    AF = mybir.ActivationFunctionType
    ALU = mybir.AluOpType

    nc = bacc.Bacc("TRN2", target_bir_lowering=False, debug=False,
                   num_devices=NCORES)

    WA = nc.dram_tensor("WA", [N, N], F16, kind="ExternalInput")    # A.T  [k, m]
    WC = nc.dram_tensor("WC", [N, N], F16, kind="ExternalInput")    # C.T  [k, m]
    BY = nc.dram_tensor("BY", [N, 1], F32, kind="ExternalInput")
    X0T = nc.dram_tensor("X0T", [N, BLOC], F32, kind="ExternalInput")
    OUT = nc.dram_tensor("OUT", [BLOC, TMAX, N], F32, kind="ExternalOutput")

    with tile.TileContext(nc) as tc:
        with (
            tc.tile_pool(name="consts", bufs=1) as consts,
            tc.tile_pool(name="xqpool", bufs=3) as xqpool,
            tc.tile_pool(name="mpool", bufs=3) as mpool,
            tc.tile_pool(name="ppool", bufs=2) as ppool,
            tc.tile_pool(name="tpool", bufs=2) as tpool,
            tc.tile_pool(name="stpool", bufs=4) as stpool,
            tc.tile_pool(name="psv", bufs=2, space="PSUM") as psv,
            tc.tile_pool(name="psu", bufs=2, space="PSUM") as psu,
            tc.tile_pool(name="pst", bufs=3, space="PSUM") as pst,
        ):
            # ---- constants / initial state ----
            wa = [[consts.tile([128, 128], F16, name=f"wa{k}{mc}", tag=f"wa{k}{mc}")
                   for mc in range(2)] for k in range(2)]
            wc = [[consts.tile([128, 128], F16, name=f"wc{k}{mc}", tag=f"wc{k}{mc}")
                   for mc in range(2)] for k in range(2)]
            for k in range(2):
                for mc in range(2):
                    nc.sync.dma_start(wa[k][mc][:], WA[128 * k:128 * (k + 1), 128 * mc:128 * (mc + 1)])
                    nc.sync.dma_start(wc[k][mc][:], WC[128 * k:128 * (k + 1), 128 * mc:128 * (mc + 1)])
            by_sb = consts.tile([128, 2], F32, tag="by")
            nc.sync.dma_start(by_sb[:, 0:1], BY[0:128, :])
            nc.sync.dma_start(by_sb[:, 1:2], BY[128:256, :])
            ident_f32 = consts.tile([128, 128], F32, tag="ident_f32")
            make_identity(nc, ident_f32[:])
            ident = consts.tile([128, 128], F16, tag="ident")
            nc.vector.tensor_copy(ident[:], ident_f32[:])

            m = mpool.tile([128, 2 * BLOC], F32, tag="m")   # master fp32 state
            nc.sync.dma_start(m[:, 0:BLOC], X0T[0:128, :])
            nc.sync.dma_start(m[:, BLOC:2 * BLOC], X0T[128:256, :])
            xq = xqpool.tile([128, 2 * BLOC], F16, tag="xq")  # fp16 copy for PE
            nc.vector.tensor_copy(xq[:], m[:])

            # ---- recurrence: M_i = M_{i-1} + STEP*(A@xq + tanh(C@xq + by)) ----
            GRP = 4  # output steps batched per stage copy/DMA
            pt = None
            pend = []
            for t in range(1, n_steps + 1):
                g = (t - 1) % GRP
                gn = min(GRP, n_steps - (t - 1 - g))  # size of this group
                pv = psv.tile([128, 2 * BLOC], F32, tag="pv")
                pu = psu.tile([128, 2 * BLOC], F32, tag="pu")
                # V = C@xq first (feeds the tanh -> chain), then U = A@xq
                for mc in range(2):
                    ms = slice(BLOC * mc, BLOC * (mc + 1))
                    nc.tensor.matmul(pv[:, ms], wc[0][mc][:], xq[:, 0:BLOC],
                                     start=True, stop=False)
                    nc.tensor.matmul(pv[:, ms], wc[1][mc][:], xq[:, BLOC:2 * BLOC],
                                     start=False, stop=True)
                for mc in range(2):
                    ms = slice(BLOC * mc, BLOC * (mc + 1))
                    nc.tensor.matmul(pu[:, ms], wa[0][mc][:], xq[:, 0:BLOC],
                                     start=True, stop=False)
                    nc.tensor.matmul(pu[:, ms], wa[1][mc][:], xq[:, BLOC:2 * BLOC],
                                     start=False, stop=True)

                # P = M + STEP*U  (off the tanh chain; runs while ACT computes tanh)
                p = ppool.tile([128, 2 * BLOC], F32, tag="p")
                nc.vector.scalar_tensor_tensor(
                    p[:], pu[:], STEP, m[:], op0=ALU.mult, op1=ALU.add)

                # tanh per m-chunk (fused per-partition bias), staggered so the
                # next-step k0 matmuls can start as soon as xq chunk0 lands
                tt = tpool.tile([128, 2 * BLOC], F32, tag="tt")
                nc.scalar.activation(tt[:, 0:BLOC], pv[:, 0:BLOC], AF.Tanh,
                                     bias=by_sb[:, 0:1], scale=1.0)
                nc.scalar.activation(tt[:, BLOC:2 * BLOC], pv[:, BLOC:2 * BLOC],
                                     AF.Tanh, bias=by_sb[:, 1:2], scale=1.0)

                # chain ops: next PE input (fp16), per chunk
                xq = xqpool.tile([128, 2 * BLOC], F16, tag="xq")
                nc.vector.scalar_tensor_tensor(
                    xq[:, 0:BLOC], tt[:, 0:BLOC], STEP, p[:, 0:BLOC],
                    op0=ALU.mult, op1=ALU.add)
                nc.vector.scalar_tensor_tensor(
                    xq[:, BLOC:2 * BLOC], tt[:, BLOC:2 * BLOC], STEP,
                    p[:, BLOC:2 * BLOC], op0=ALU.mult, op1=ALU.add)
                # master state, same math in fp32 (off chain)
                m = mpool.tile([128, 2 * BLOC], F32, tag="m")
                nc.vector.scalar_tensor_tensor(
                    m[:], tt[:], STEP, p[:], op0=ALU.mult, op1=ALU.add)

                # output row t: transpose state copy [128, 2b] -> [b, 256]
                # into a GRP-step PSUM batch; emitted one iteration late so the
                # next step's V matmuls outrank the transposes when xq lands
                pend.append((t, g, gn, xq))
                if len(pend) == 2 or t == n_steps:
                    for (tp, gp, gnp, xqp) in (pend if t == n_steps else pend[:1]):
                        if gp == 0:
                            pt = pst.tile([BLOC, GRP, N], F16, tag="pt", name="pt")
                        nc.tensor.transpose(pt[:, gp, 0:128], xqp[:, 0:BLOC], ident[:])
                        nc.tensor.transpose(pt[:, gp, 128:256], xqp[:, BLOC:2 * BLOC], ident[:])
                        if gp == gnp - 1:
                            stage = stpool.tile([BLOC, GRP, N], F32, tag="stage", name="stage")
                            half = (gnp + 1) // 2
                            nc.vector.tensor_copy(stage[:, 0:half], pt[:, 0:half])
                            nc.scalar.copy(stage[:, half:gnp], pt[:, half:gnp])
                            nc.sync.dma_start(OUT[:, tp - gnp + 1:tp + 1, :], stage[:, 0:gnp])
                    pend = pend[1:] if t != n_steps else []
    nc.compile()
    return nc


def kernel(X0, MA, MW, bA_z, bW_z, by_w):
    global LAST_RESULT
    from concourse.bass_utils import run_bass_kernel_spmd

    X0 = np.asarray(X0, dtype=np.float32)
    MA = np.asarray(MA, dtype=np.float32)
    MW = np.asarray(MW, dtype=np.float32)
    bA_z = np.asarray(bA_z, dtype=np.float32)
    bW_z = np.asarray(bW_z, dtype=np.float32)
    by_w = np.asarray(by_w, dtype=np.float32)

    # host-side weight prep (f32, matches reference math); weights to fp16
    # for full-rate PE matmuls (master state stays fp32 on device).
    bA = np.float32(0.5) * np.exp(-bA_z[0, 0] * bA_z[0, 0]) + np.float32(0.5)
    bW = np.float32(0.5) * np.exp(-bW_z[0, 0] * bW_z[0, 0]) + np.float32(0.5)
    I = np.eye(N, dtype=np.float32)
    A = (1 - bA) * (MA + MA.T) + bA * (MA - MA.T) - np.float32(YA) * I
    C = (1 - bA) * (MW + MW.T) + bW * (MW - MW.T) - np.float32(YW) * I
    WA = np.ascontiguousarray(A.T).astype(np.float16)
    WC = np.ascontiguousarray(C.T).astype(np.float16)

    n_steps = TMAX - 1
    in_maps = []
    for i in range(NCORES):
        in_maps.append({
            "WA": WA,
            "WC": WC,
            "BY": by_w,
            "X0T": np.ascontiguousarray(X0[i * BLOC:(i + 1) * BLOC, :].T),
        })

    nc = _build(n_steps)
    res = run_bass_kernel_spmd(nc, in_maps, core_ids=list(range(NCORES)))
    LAST_RESULT = res

    out = np.concatenate([r["OUT"] for r in res.results], axis=0)
    out[:, 0, :] = X0
    return out


if __name__ == "__main__":
    rng = np.random.default_rng(0)
    inputs = {
        "X0": rng.standard_normal((BS, N), dtype=np.float32),
        "MA": rng.standard_normal((N, N), dtype=np.float32) / 16,
        "MW": rng.standard_normal((N, N), dtype=np.float32) / 16,
        "bA_z": np.full((1, 1), 0.65, dtype=np.float32),
        "bW_z": np.full((1, 1), 0.65, dtype=np.float32),
        "by_w": rng.standard_normal((N, 1), dtype=np.float32) / 100,
    }
    out = kernel(**inputs)
    print("out", out.shape, out.dtype, np.abs(out).max())


# revision 19
# speedup vs baseline: 1.2033x; 1.2033x over previous
"""LipschitzRNN Trainium2 kernel.

Math (per reference):
    bA = 0.5*exp(-bA_z^2)+0.5 ; bW likewise
    A = (1-bA)(MA+MA.T) + bA(MA-MA.T) - YA*I
    C = (1-bA)(MW+MW.T) + bW(MW-MW.T) - YW*I
    X_{t+1} = X_t + STEP*(A@X_t + tanh(C@X_t + by))   (column-state X: [n, bs])
    out[b, t, :] = X_t[:, b]

Device strategy (8-way batch data-parallel, b=32/core, no collectives):
  - State kept as [n(partitions), b] in SBUF: two k-chunks side by side
    [128, 64], so elementwise ops use all 128 partitions, the per-partition
    bias `by` fuses into ScalarE's tanh, and no per-step transpose is needed
    for the recurrence itself.
  - Matmul operands in fp16 (FWL-eligible weight loads, single-pass PE).
    A fp32 "master" state is kept alongside the fp16 copy, with the
    UNFOLDED update  X' = X + STEP*(A@xq + tanh(C@xq + by)) , so fp16
    quantization of weights/state only enters through STEP-scaled paths
    (measured end-to-end relative error ~9e-4 over 511 steps).
  - Per step: 8 matmuls (V=C@xq first -> feeds tanh chain; U=A@xq), then
    P = X + STEP*U on VectorE while ScalarE runs the two biased tanh chunks,
    then xq' (fp16) and X' (fp32) = (tanh*STEP)+P.
  - Output rows need [b, n] layout: PE-transposes of xq (emitted one
    iteration late so next-step matmuls win scheduler priority), batched
    4 steps per SBUF stage copy (split between VectorE and ScalarE) and
    per 128KB DMA.
"""

import os
import numpy as np

N = 256
BS = 256
TMAX = 512
STEP = 0.01
YA = 0.001
YW = 0.001
NCORES = 8
BLOC = BS // NCORES  # 32

LAST_RESULT = None  # BassKernelResults of the most recent run (for test harness)


def _build(n_steps):
    from concourse import bacc, tile
    import concourse.mybir as mybir
    from concourse.masks import make_identity

    F32 = mybir.dt.float32
    F16 = mybir.dt.float16
    AF = mybir.ActivationFunctionType
    ALU = mybir.AluOpType

    nc = bacc.Bacc("TRN2", target_bir_lowering=False, debug=False,
                   num_devices=NCORES)

    WA = nc.dram_tensor("WA", [N, N], F16, kind="ExternalInput")    # A.T  [k, m]
    WC = nc.dram_tensor("WC", [N, N], F16, kind="ExternalInput")    # C.T  [k, m]
    BY = nc.dram_tensor("BY", [N, 1], F32, kind="ExternalInput")
    X0T = nc.dram_tensor("X0T", [N, BLOC], F32, kind="ExternalInput")
    OUT = nc.dram_tensor("OUT", [BLOC, TMAX, N], F32, kind="ExternalOutput")

    with tile.TileContext(nc) as tc:
        with (
            tc.tile_pool(name="consts", bufs=1) as consts,
            tc.tile_pool(name="xqpool", bufs=3) as xqpool,
            tc.tile_pool(name="mpool", bufs=3) as mpool,
            tc.tile_pool(name="ppool", bufs=2) as ppool,
            tc.tile_pool(name="tpool", bufs=2) as tpool,
            tc.tile_pool(name="stpool", bufs=4) as stpool,
            tc.tile_pool(name="psv0", bufs=2, space="PSUM") as psv0,
            tc.tile_pool(name="psv1", bufs=2, space="PSUM") as psv1,
            tc.tile_pool(name="psu", bufs=2, space="PSUM") as psu,
            tc.tile_pool(name="pst", bufs=2, space="PSUM") as pst,
        ):
            # ---- constants / initial state ----
            wa = [[consts.tile([128, 128], F16, name=f"wa{k}{mc}", tag=f"wa{k}{mc}")
                   for mc in range(2)] for k in range(2)]
            wc = [[consts.tile([128, 128], F16, name=f"wc{k}{mc}", tag=f"wc{k}{mc}")
                   for mc in range(2)] for k in range(2)]
            for k in range(2):
                for mc in range(2):
                    nc.sync.dma_start(wa[k][mc][:], WA[128 * k:128 * (k + 1), 128 * mc:128 * (mc + 1)])
                    nc.sync.dma_start(wc[k][mc][:], WC[128 * k:128 * (k + 1), 128 * mc:128 * (mc + 1)])
            by_sb = consts.tile([128, 2], F32, tag="by")
            nc.sync.dma_start(by_sb[:, 0:1], BY[0:128, :])
            nc.sync.dma_start(by_sb[:, 1:2], BY[128:256, :])
            ident_f32 = consts.tile([128, 128], F32, tag="ident_f32")
            make_identity(nc, ident_f32[:])
            ident = consts.tile([128, 128], F16, tag="ident")
            nc.vector.tensor_copy(ident[:], ident_f32[:])

            m = mpool.tile([128, 2 * BLOC], F32, tag="m")   # master fp32 state
            nc.sync.dma_start(m[:, 0:BLOC], X0T[0:128, :])
            nc.sync.dma_start(m[:, BLOC:2 * BLOC], X0T[128:256, :])
            xq0 = xqpool.tile([128, BLOC], F16, tag="xq0", name="xq0")
            xq1 = xqpool.tile([128, BLOC], F16, tag="xq1", name="xq1")
            nc.vector.tensor_copy(xq0[:], m[:, 0:BLOC])
            nc.vector.tensor_copy(xq1[:], m[:, BLOC:2 * BLOC])

            # ---- recurrence: M_i = M_{i-1} + STEP*(A@xq + tanh(C@xq + by)) ----
            GRP = 4  # output steps batched per stage copy/DMA
            pt = None
            pend = []
            for t in range(1, n_steps + 1):
                g = (t - 1) % GRP
                gn = min(GRP, n_steps - (t - 1 - g))  # size of this group
                pv = [psv0.tile([128, BLOC], F32, tag="pv0", name="pv0"),
                      psv1.tile([128, BLOC], F32, tag="pv1", name="pv1")]
                pu = psu.tile([128, 2 * BLOC], F32, tag="pu")
                # V = C@xq first (feeds the tanh -> chain), then U = A@xq.
                # Separate pv/xq tiles per chunk: tanh0 only waits its 2 matmuls.
                for mc in range(2):
                    nc.tensor.matmul(pv[mc][:], wc[0][mc][:], xq0[:],
                                     start=True, stop=False)
                    nc.tensor.matmul(pv[mc][:], wc[1][mc][:], xq1[:],
                                     start=False, stop=True)
                for mc in range(2):
                    ms = slice(BLOC * mc, BLOC * (mc + 1))
                    nc.tensor.matmul(pu[:, ms], wa[0][mc][:], xq0[:],
                                     start=True, stop=False)
                    nc.tensor.matmul(pu[:, ms], wa[1][mc][:], xq1[:],
                                     start=False, stop=True)

                # P = M + STEP*U  (off the tanh chain; runs while ACT computes tanh)
                p = ppool.tile([128, 2 * BLOC], F32, tag="p")
                nc.vector.scalar_tensor_tensor(
                    p[:], pu[:], STEP, m[:], op0=ALU.mult, op1=ALU.add)

                # tanh per m-chunk (fused per-partition bias), staggered so the
                # next-step k0 matmuls can start as soon as xq chunk0 lands
                tt0 = tpool.tile([128, BLOC], F32, tag="tt0", name="tt0")
                tt1 = tpool.tile([128, BLOC], F32, tag="tt1", name="tt1")
                nc.scalar.activation(tt0[:], pv[0][:], AF.Tanh,
                                     bias=by_sb[:, 0:1], scale=1.0)
                nc.scalar.activation(tt1[:], pv[1][:], AF.Tanh,
                                     bias=by_sb[:, 1:2], scale=1.0)

                # chain ops: next PE input (fp16), per chunk
                xq0 = xqpool.tile([128, BLOC], F16, tag="xq0", name="xq0")
                xq1 = xqpool.tile([128, BLOC], F16, tag="xq1", name="xq1")
                nc.vector.scalar_tensor_tensor(
                    xq0[:], tt0[:], STEP, p[:, 0:BLOC],
                    op0=ALU.mult, op1=ALU.add)
                nc.vector.scalar_tensor_tensor(
                    xq1[:], tt1[:], STEP, p[:, BLOC:2 * BLOC],
                    op0=ALU.mult, op1=ALU.add)
                # master state, same math in fp32 (off chain), per chunk
                m = mpool.tile([128, 2 * BLOC], F32, tag="m")
                nc.vector.scalar_tensor_tensor(
                    m[:, 0:BLOC], tt0[:], STEP, p[:, 0:BLOC],
                    op0=ALU.mult, op1=ALU.add)
                nc.vector.scalar_tensor_tensor(
                    m[:, BLOC:2 * BLOC], tt1[:], STEP, p[:, BLOC:2 * BLOC],
                    op0=ALU.mult, op1=ALU.add)

                # output row t: transpose state copy [128, 2b] -> [b, 256]
                # into a GRP-step PSUM batch; emitted one iteration late so the
                # next step's V matmuls outrank the transposes when xq lands
                pend.append((t, g, gn, xq0, xq1))
                if len(pend) == 2 or t == n_steps:
                    for (tp, gp, gnp, xqp0, xqp1) in (pend if t == n_steps else pend[:1]):
                        if gp == 0:
                            pt = pst.tile([BLOC, GRP, N], F16, tag="pt", name="pt")
                        nc.tensor.transpose(pt[:, gp, 0:128], xqp0[:], ident[:])
                        nc.tensor.transpose(pt[:, gp, 128:256], xqp1[:], ident[:])
                        if gp == gnp - 1:
                            stage = stpool.tile([BLOC, GRP, N], F32, tag="stage", name="stage")
                            half = (gnp + 1) // 2
                            nc.vector.tensor_copy(stage[:, 0:half], pt[:, 0:half])
                            nc.scalar.copy(stage[:, half:gnp], pt[:, half:gnp])
                            nc.sync.dma_start(OUT[:, tp - gnp + 1:tp + 1, :], stage[:, 0:gnp])
                    pend = pend[1:] if t != n_steps else []
    nc.compile()
    return nc


def kernel(X0, MA, MW, bA_z, bW_z, by_w):
    global LAST_RESULT
    from concourse.bass_utils import run_bass_kernel_spmd

    X0 = np.asarray(X0, dtype=np.float32)
    MA = np.asarray(MA, dtype=np.float32)
    MW = np.asarray(MW, dtype=np.float32)
    bA_z = np.asarray(bA_z, dtype=np.float32)
    bW_z = np.asarray(bW_z, dtype=np.float32)
    by_w = np.asarray(by_w, dtype=np.float32)

    # host-side weight prep (f32, matches reference math); weights to fp16
    # for full-rate PE matmuls (master state stays fp32 on device).
    bA = np.float32(0.5) * np.exp(-bA_z[0, 0] * bA_z[0, 0]) + np.float32(0.5)
    bW = np.float32(0.5) * np.exp(-bW_z[0, 0] * bW_z[0, 0]) + np.float32(0.5)
    I = np.eye(N, dtype=np.float32)
    A = (1 - bA) * (MA + MA.T) + bA * (MA - MA.T) - np.float32(YA) * I
    C = (1 - bA) * (MW + MW.T) + bW * (MW - MW.T) - np.float32(YW) * I
    WA = np.ascontiguousarray(A.T).astype(np.float16)
    WC = np.ascontiguousarray(C.T).astype(np.float16)

    n_steps = TMAX - 1
    in_maps = []
    for i in range(NCORES):
        in_maps.append({
            "WA": WA,
            "WC": WC,
            "BY": by_w,
            "X0T": np.ascontiguousarray(X0[i * BLOC:(i + 1) * BLOC, :].T),
        })

    nc = _build(n_steps)
    res = run_bass_kernel_spmd(nc, in_maps, core_ids=list(range(NCORES)))
    LAST_RESULT = res

    out = np.concatenate([r["OUT"] for r in res.results], axis=0)
    out[:, 0, :] = X0
    return out


if __name__ == "__main__":
    rng = np.random.default_rng(0)
    inputs = {
        "X0": rng.standard_normal((BS, N), dtype=np.float32),
        "MA": rng.standard_normal((N, N), dtype=np.float32) / 16,
        "MW": rng.standard_normal((N, N), dtype=np.float32) / 16,
        "bA_z": np.full((1, 1), 0.65, dtype=np.float32),
        "bW_z": np.full((1, 1), 0.65, dtype=np.float32),
        "by_w": rng.standard_normal((N, 1), dtype=np.float32) / 100,
    }
    out = kernel(**inputs)
    print("out", out.shape, out.dtype, np.abs(out).max())


# revision 20
# speedup vs baseline: 1.2043x; 1.0008x over previous
"""LipschitzRNN Trainium2 kernel.

Math (per reference):
    bA = 0.5*exp(-bA_z^2)+0.5 ; bW likewise
    A = (1-bA)(MA+MA.T) + bA(MA-MA.T) - YA*I
    C = (1-bA)(MW+MW.T) + bW(MW-MW.T) - YW*I
    X_{t+1} = X_t + STEP*(A@X_t + tanh(C@X_t + by))   (column-state X: [n, bs])
    out[b, t, :] = X_t[:, b]

Device strategy (8-way batch data-parallel, b=32/core, no collectives):
  - State kept as [n(partitions), b] in SBUF: two k-chunks side by side
    [128, 64], so elementwise ops use all 128 partitions, the per-partition
    bias `by` fuses into ScalarE's tanh, and no per-step transpose is needed
    for the recurrence itself.
  - Matmul operands in fp16 (FWL-eligible weight loads, single-pass PE).
    A fp32 "master" state is kept alongside the fp16 copy, with the
    UNFOLDED update  X' = X + STEP*(A@xq + tanh(C@xq + by)) , so fp16
    quantization of weights/state only enters through STEP-scaled paths
    (measured end-to-end relative error ~9e-4 over 511 steps).
  - Per step: 8 matmuls (V=C@xq first -> feeds tanh chain; U=A@xq), then
    P = X + STEP*U on VectorE while ScalarE runs the two biased tanh chunks,
    then xq' (fp16) and X' (fp32) = (tanh*STEP)+P.
  - Output rows need [b, n] layout: PE-transposes of xq (emitted one
    iteration late so next-step matmuls win scheduler priority), batched
    4 steps per SBUF stage copy (split between VectorE and ScalarE) and
    per 128KB DMA.
"""

import os
import numpy as np

N = 256
BS = 256
TMAX = 512
STEP = 0.01
YA = 0.001
YW = 0.001
NCORES = 8
BLOC = BS // NCORES  # 32

LAST_RESULT = None  # BassKernelResults of the most recent run (for test harness)


def _build(n_steps):
    from concourse import bacc, tile
    import concourse.mybir as mybir
    from concourse.masks import make_identity

    F32 = mybir.dt.float32
    F16 = mybir.dt.float16
    AF = mybir.ActivationFunctionType
    ALU = mybir.AluOpType

    nc = bacc.Bacc("TRN2", target_bir_lowering=False, debug=False,
                   num_devices=NCORES)

    WA = nc.dram_tensor("WA", [N, N], F16, kind="ExternalInput")    # A.T  [k, m]
    WC = nc.dram_tensor("WC", [N, N], F16, kind="ExternalInput")    # C.T  [k, m]
    BY = nc.dram_tensor("BY", [N, 1], F32, kind="ExternalInput")
    X0T = nc.dram_tensor("X0T", [N, BLOC], F32, kind="ExternalInput")
    OUT = nc.dram_tensor("OUT", [BLOC, TMAX, N], F32, kind="ExternalOutput")

    with tile.TileContext(nc) as tc:
        with (
            tc.tile_pool(name="consts", bufs=1) as consts,
            tc.tile_pool(name="xqpool", bufs=3) as xqpool,
            tc.tile_pool(name="mpool", bufs=3) as mpool,
            tc.tile_pool(name="ppool", bufs=2) as ppool,
            tc.tile_pool(name="tpool", bufs=2) as tpool,
            tc.tile_pool(name="stpool", bufs=4) as stpool,
            tc.tile_pool(name="psv0", bufs=2, space="PSUM") as psv0,
            tc.tile_pool(name="psv1", bufs=2, space="PSUM") as psv1,
            tc.tile_pool(name="psu", bufs=2, space="PSUM") as psu,
            tc.tile_pool(name="pst", bufs=2, space="PSUM") as pst,
        ):
            # ---- constants / initial state ----
            wa = [[consts.tile([128, 128], F16, name=f"wa{k}{mc}", tag=f"wa{k}{mc}")
                   for mc in range(2)] for k in range(2)]
            wc = [[consts.tile([128, 128], F16, name=f"wc{k}{mc}", tag=f"wc{k}{mc}")
                   for mc in range(2)] for k in range(2)]
            for k in range(2):
                for mc in range(2):
                    nc.sync.dma_start(wa[k][mc][:], WA[128 * k:128 * (k + 1), 128 * mc:128 * (mc + 1)])
                    nc.sync.dma_start(wc[k][mc][:], WC[128 * k:128 * (k + 1), 128 * mc:128 * (mc + 1)])
            by_sb = consts.tile([128, 2], F32, tag="by")
            nc.sync.dma_start(by_sb[:, 0:1], BY[0:128, :])
            nc.sync.dma_start(by_sb[:, 1:2], BY[128:256, :])
            ident_f32 = consts.tile([128, 128], F32, tag="ident_f32")
            make_identity(nc, ident_f32[:])
            ident = consts.tile([128, 128], F16, tag="ident")
            nc.vector.tensor_copy(ident[:], ident_f32[:])

            m = mpool.tile([128, 2 * BLOC], F32, tag="m")   # master fp32 state
            nc.sync.dma_start(m[:, 0:BLOC], X0T[0:128, :])
            nc.sync.dma_start(m[:, BLOC:2 * BLOC], X0T[128:256, :])
            xq0 = xqpool.tile([128, BLOC], F16, tag="xq0", name="xq0")
            xq1 = xqpool.tile([128, BLOC], F16, tag="xq1", name="xq1")
            nc.vector.tensor_copy(xq0[:], m[:, 0:BLOC])
            nc.vector.tensor_copy(xq1[:], m[:, BLOC:2 * BLOC])

            # ---- recurrence: M_i = M_{i-1} + STEP*(A@xq + tanh(C@xq + by)) ----
            GRP = 4  # output steps batched per stage copy/DMA
            pt = None
            pend = []
            pend2 = []
            for t in range(1, n_steps + 1):
                g = (t - 1) % GRP
                gn = min(GRP, n_steps - (t - 1 - g))  # size of this group
                pv = [psv0.tile([128, BLOC], F32, tag="pv0", name="pv0"),
                      psv1.tile([128, BLOC], F32, tag="pv1", name="pv1")]
                pu = psu.tile([128, 2 * BLOC], F32, tag="pu")
                # V = C@xq first (feeds the tanh -> chain), then U = A@xq.
                # Separate pv/xq tiles per chunk: tanh0 only waits its 2 matmuls.
                for mc in range(2):
                    nc.tensor.matmul(pv[mc][:], wc[0][mc][:], xq0[:],
                                     start=True, stop=False)
                    nc.tensor.matmul(pv[mc][:], wc[1][mc][:], xq1[:],
                                     start=False, stop=True)
                for mc in range(2):
                    ms = slice(BLOC * mc, BLOC * (mc + 1))
                    nc.tensor.matmul(pu[:, ms], wa[0][mc][:], xq0[:],
                                     start=True, stop=False)
                    nc.tensor.matmul(pu[:, ms], wa[1][mc][:], xq1[:],
                                     start=False, stop=True)

                # P = M + STEP*U  (off the tanh chain; runs while ACT computes tanh)
                p = ppool.tile([128, 2 * BLOC], F32, tag="p")
                nc.vector.scalar_tensor_tensor(
                    p[:], pu[:], STEP, m[:], op0=ALU.mult, op1=ALU.add)

                # tanh per m-chunk (fused per-partition bias), staggered so the
                # next-step k0 matmuls can start as soon as xq chunk0 lands
                tt0 = tpool.tile([128, BLOC], F32, tag="tt0", name="tt0")
                tt1 = tpool.tile([128, BLOC], F32, tag="tt1", name="tt1")
                nc.scalar.activation(tt0[:], pv[0][:], AF.Tanh,
                                     bias=by_sb[:, 0:1], scale=1.0)
                nc.scalar.activation(tt1[:], pv[1][:], AF.Tanh,
                                     bias=by_sb[:, 1:2], scale=1.0)

                # chain ops: next PE input (fp16), per chunk
                xq0 = xqpool.tile([128, BLOC], F16, tag="xq0", name="xq0")
                xq1 = xqpool.tile([128, BLOC], F16, tag="xq1", name="xq1")
                nc.vector.scalar_tensor_tensor(
                    xq0[:], tt0[:], STEP, p[:, 0:BLOC],
                    op0=ALU.mult, op1=ALU.add)
                nc.vector.scalar_tensor_tensor(
                    xq1[:], tt1[:], STEP, p[:, BLOC:2 * BLOC],
                    op0=ALU.mult, op1=ALU.add)
                # master state, same math in fp32 (off chain), per chunk
                m = mpool.tile([128, 2 * BLOC], F32, tag="m")
                nc.vector.scalar_tensor_tensor(
                    m[:, 0:BLOC], tt0[:], STEP, p[:, 0:BLOC],
                    op0=ALU.mult, op1=ALU.add)
                nc.vector.scalar_tensor_tensor(
                    m[:, BLOC:2 * BLOC], tt1[:], STEP, p[:, BLOC:2 * BLOC],
                    op0=ALU.mult, op1=ALU.add)

                # output row t: transpose state copy [128, 2b] -> [b, 256]
                # into a GRP-step PSUM batch; emitted one iteration late so the
                # next step's V matmuls outrank the transposes when xq lands
                pend.append((t, g, gn, xq0, xq1))
                if len(pend) == 2 or t == n_steps:
                    for (tp, gp, gnp, xqp0, xqp1) in (pend if t == n_steps else pend[:1]):
                        if gp == 0:
                            pt = pst.tile([BLOC, GRP, N], F16, tag="pt", name="pt")
                        nc.tensor.transpose(pt[:, gp, 0:128], xqp0[:], ident[:])
                        nc.tensor.transpose(pt[:, gp, 128:256], xqp1[:], ident[:])
                        if gp == gnp - 1:
                            pend2.append((tp, gnp, pt))
                    pend = [] if t == n_steps else pend[1:]
                # stage copies one extra iteration late, so their sem waits are
                # already satisfied when they enter the DVE/ACT queues
                if t == n_steps:
                    ready, pend2 = pend2, []
                else:
                    ready = [e for e in pend2 if e[0] <= t - 2]
                    pend2 = [e for e in pend2 if e[0] > t - 2]
                for (tp, gnp, ptp) in ready:
                    stage = stpool.tile([BLOC, GRP, N], F32, tag="stage", name="stage")
                    half = (gnp + 1) // 2
                    nc.vector.tensor_copy(stage[:, 0:half], ptp[:, 0:half])
                    nc.scalar.copy(stage[:, half:gnp], ptp[:, half:gnp])
                    nc.sync.dma_start(OUT[:, tp - gnp + 1:tp + 1, :], stage[:, 0:gnp])
    nc.compile()
    return nc


def kernel(X0, MA, MW, bA_z, bW_z, by_w):
    global LAST_RESULT
    from concourse.bass_utils import run_bass_kernel_spmd

    X0 = np.asarray(X0, dtype=np.float32)
    MA = np.asarray(MA, dtype=np.float32)
    MW = np.asarray(MW, dtype=np.float32)
    bA_z = np.asarray(bA_z, dtype=np.float32)
    bW_z = np.asarray(bW_z, dtype=np.float32)
    by_w = np.asarray(by_w, dtype=np.float32)

    # host-side weight prep (f32, matches reference math); weights to fp16
    # for full-rate PE matmuls (master state stays fp32 on device).
    bA = np.float32(0.5) * np.exp(-bA_z[0, 0] * bA_z[0, 0]) + np.float32(0.5)
    bW = np.float32(0.5) * np.exp(-bW_z[0, 0] * bW_z[0, 0]) + np.float32(0.5)
    I = np.eye(N, dtype=np.float32)
    A = (1 - bA) * (MA + MA.T) + bA * (MA - MA.T) - np.float32(YA) * I
    C = (1 - bA) * (MW + MW.T) + bW * (MW - MW.T) - np.float32(YW) * I
    WA = np.ascontiguousarray(A.T).astype(np.float16)
    WC = np.ascontiguousarray(C.T).astype(np.float16)

    n_steps = TMAX - 1
    in_maps = []
    for i in range(NCORES):
        in_maps.append({
            "WA": WA,
            "WC": WC,
            "BY": by_w,
            "X0T": np.ascontiguousarray(X0[i * BLOC:(i + 1) * BLOC, :].T),
        })

    nc = _build(n_steps)
    res = run_bass_kernel_spmd(nc, in_maps, core_ids=list(range(NCORES)))
    LAST_RESULT = res

    out = np.concatenate([r["OUT"] for r in res.results], axis=0)
    out[:, 0, :] = X0
    return out


if __name__ == "__main__":
    rng = np.random.default_rng(0)
    inputs = {
        "X0": rng.standard_normal((BS, N), dtype=np.float32),
        "MA": rng.standard_normal((N, N), dtype=np.float32) / 16,
        "MW": rng.standard_normal((N, N), dtype=np.float32) / 16,
        "bA_z": np.full((1, 1), 0.65, dtype=np.float32),
        "bW_z": np.full((1, 1), 0.65, dtype=np.float32),
        "by_w": rng.standard_normal((N, 1), dtype=np.float32) / 100,
    }
    out = kernel(**inputs)
    print("out", out.shape, out.dtype, np.abs(out).max())


# revision 21
# speedup vs baseline: 1.2131x; 1.0073x over previous
"""LipschitzRNN Trainium2 kernel.

Math (per reference):
    bA = 0.5*exp(-bA_z^2)+0.5 ; bW likewise
    A = (1-bA)(MA+MA.T) + bA(MA-MA.T) - YA*I
    C = (1-bA)(MW+MW.T) + bW(MW-MW.T) - YW*I
    X_{t+1} = X_t + STEP*(A@X_t + tanh(C@X_t + by))   (column-state X: [n, bs])
    out[b, t, :] = X_t[:, b]

Device strategy (8-way batch data-parallel, b=32/core, no collectives):
  - State kept as [n(partitions), b] in SBUF: two k-chunks side by side
    [128, 64], so elementwise ops use all 128 partitions, the per-partition
    bias `by` fuses into ScalarE's tanh, and no per-step transpose is needed
    for the recurrence itself.
  - Matmul operands in fp16 (FWL-eligible weight loads, single-pass PE).
    A fp32 "master" state is kept alongside the fp16 copy, with the
    UNFOLDED update  X' = X + STEP*(A@xq + tanh(C@xq + by)) , so fp16
    quantization of weights/state only enters through STEP-scaled paths
    (measured end-to-end relative error ~9e-4 over 511 steps).
  - Per step: 8 matmuls (V=C@xq first -> feeds tanh chain; U=A@xq), then
    P = X + STEP*U on VectorE while ScalarE runs the two biased tanh chunks,
    then xq' (fp16) and X' (fp32) = (tanh*STEP)+P.
  - Output rows need [b, n] layout: PE-transposes of xq (emitted one
    iteration late so next-step matmuls win scheduler priority), batched
    4 steps per SBUF stage copy (split between VectorE and ScalarE) and
    per 128KB DMA.
"""

import os
import numpy as np

N = 256
BS = 256
TMAX = 512
STEP = 0.01
YA = 0.001
YW = 0.001
NCORES = 8
BLOC = BS // NCORES  # 32

LAST_RESULT = None  # BassKernelResults of the most recent run (for test harness)


def _build(n_steps):
    from concourse import bacc, tile
    import concourse.mybir as mybir
    from concourse.masks import make_identity

    F32 = mybir.dt.float32
    F16 = mybir.dt.float16
    AF = mybir.ActivationFunctionType
    ALU = mybir.AluOpType

    nc = bacc.Bacc("TRN2", target_bir_lowering=False, debug=False,
                   num_devices=NCORES)

    WA = nc.dram_tensor("WA", [N, N], F16, kind="ExternalInput")    # A.T  [k, m]
    WC = nc.dram_tensor("WC", [N, N], F16, kind="ExternalInput")    # C.T  [k, m]
    BY = nc.dram_tensor("BY", [N, 1], F32, kind="ExternalInput")
    X0T = nc.dram_tensor("X0T", [N, BLOC], F32, kind="ExternalInput")
    OUT = nc.dram_tensor("OUT", [BLOC, TMAX, N], F32, kind="ExternalOutput")

    with tile.TileContext(nc) as tc:
        with (
            tc.tile_pool(name="consts", bufs=1) as consts,
            tc.tile_pool(name="xqpool", bufs=3) as xqpool,
            tc.tile_pool(name="mpool", bufs=3) as mpool,
            tc.tile_pool(name="ppool", bufs=2) as ppool,
            tc.tile_pool(name="tpool", bufs=2) as tpool,
            tc.tile_pool(name="stpool", bufs=4) as stpool,
            tc.tile_pool(name="psv0", bufs=2, space="PSUM") as psv0,
            tc.tile_pool(name="psv1", bufs=2, space="PSUM") as psv1,
            tc.tile_pool(name="psu0", bufs=1, space="PSUM") as psu0,
            tc.tile_pool(name="psu1", bufs=1, space="PSUM") as psu1,
            tc.tile_pool(name="pst", bufs=2, space="PSUM") as pst,
        ):
            # ---- constants / initial state ----
            wa = [[consts.tile([128, 128], F16, name=f"wa{k}{mc}", tag=f"wa{k}{mc}")
                   for mc in range(2)] for k in range(2)]
            wc = [[consts.tile([128, 128], F16, name=f"wc{k}{mc}", tag=f"wc{k}{mc}")
                   for mc in range(2)] for k in range(2)]
            for k in range(2):
                for mc in range(2):
                    nc.sync.dma_start(wa[k][mc][:], WA[128 * k:128 * (k + 1), 128 * mc:128 * (mc + 1)])
                    nc.sync.dma_start(wc[k][mc][:], WC[128 * k:128 * (k + 1), 128 * mc:128 * (mc + 1)])
            by_sb = consts.tile([128, 2], F32, tag="by")
            nc.sync.dma_start(by_sb[:, 0:1], BY[0:128, :])
            nc.sync.dma_start(by_sb[:, 1:2], BY[128:256, :])
            ident_f32 = consts.tile([128, 128], F32, tag="ident_f32")
            make_identity(nc, ident_f32[:])
            ident = consts.tile([128, 128], F16, tag="ident")
            nc.vector.tensor_copy(ident[:], ident_f32[:])

            m = mpool.tile([128, 2 * BLOC], F32, tag="m")   # master fp32 state
            nc.sync.dma_start(m[:, 0:BLOC], X0T[0:128, :])
            nc.sync.dma_start(m[:, BLOC:2 * BLOC], X0T[128:256, :])
            xq0 = xqpool.tile([128, BLOC], F16, tag="xq0", name="xq0")
            xq1 = xqpool.tile([128, BLOC], F16, tag="xq1", name="xq1")
            nc.vector.tensor_copy(xq0[:], m[:, 0:BLOC])
            nc.vector.tensor_copy(xq1[:], m[:, BLOC:2 * BLOC])

            # ---- recurrence: M_i = M_{i-1} + STEP*(A@xq + tanh(C@xq + by)) ----
            GRP = 4  # output steps batched per stage copy/DMA
            pt = None
            pend = []
            pend2 = []
            for t in range(1, n_steps + 1):
                g = (t - 1) % GRP
                gn = min(GRP, n_steps - (t - 1 - g))  # size of this group
                pv = [psv0.tile([128, BLOC], F32, tag="pv0", name="pv0"),
                      psv1.tile([128, BLOC], F32, tag="pv1", name="pv1")]
                pu = [psu0.tile([128, BLOC], F32, tag="pu0", name="pu0"),
                      psu1.tile([128, BLOC], F32, tag="pu1", name="pu1")]
                # Per-chunk PSUM groups, chunk-0 work first: tanh0 and p0 both
                # become ready after only 4 matmuls.
                for mc in range(2):
                    nc.tensor.matmul(pv[mc][:], wc[0][mc][:], xq0[:],
                                     start=True, stop=False)
                    nc.tensor.matmul(pv[mc][:], wc[1][mc][:], xq1[:],
                                     start=False, stop=True)
                    nc.tensor.matmul(pu[mc][:], wa[0][mc][:], xq0[:],
                                     start=True, stop=False)
                    nc.tensor.matmul(pu[mc][:], wa[1][mc][:], xq1[:],
                                     start=False, stop=True)

                # P = M + STEP*U per chunk (off the tanh chain)
                p = ppool.tile([128, 2 * BLOC], F32, tag="p")
                nc.vector.scalar_tensor_tensor(
                    p[:, 0:BLOC], pu[0][:], STEP, m[:, 0:BLOC],
                    op0=ALU.mult, op1=ALU.add)

                # tanh per m-chunk (fused per-partition bias), staggered so the
                # next-step k0 matmuls can start as soon as xq chunk0 lands
                tt0 = tpool.tile([128, BLOC], F32, tag="tt0", name="tt0")
                tt1 = tpool.tile([128, BLOC], F32, tag="tt1", name="tt1")
                nc.scalar.activation(tt0[:], pv[0][:], AF.Tanh,
                                     bias=by_sb[:, 0:1], scale=1.0)
                nc.scalar.activation(tt1[:], pv[1][:], AF.Tanh,
                                     bias=by_sb[:, 1:2], scale=1.0)

                # chain ops: next PE input (fp16), per chunk; p1 emitted
                # between them so xq0 only queues behind p0 on VectorE
                xq0 = xqpool.tile([128, BLOC], F16, tag="xq0", name="xq0")
                xq1 = xqpool.tile([128, BLOC], F16, tag="xq1", name="xq1")
                nc.vector.scalar_tensor_tensor(
                    xq0[:], tt0[:], STEP, p[:, 0:BLOC],
                    op0=ALU.mult, op1=ALU.add)
                nc.vector.scalar_tensor_tensor(
                    p[:, BLOC:2 * BLOC], pu[1][:], STEP, m[:, BLOC:2 * BLOC],
                    op0=ALU.mult, op1=ALU.add)
                nc.vector.scalar_tensor_tensor(
                    xq1[:], tt1[:], STEP, p[:, BLOC:2 * BLOC],
                    op0=ALU.mult, op1=ALU.add)
                # master state, same math in fp32 (off chain), per chunk
                m = mpool.tile([128, 2 * BLOC], F32, tag="m")
                nc.vector.scalar_tensor_tensor(
                    m[:, 0:BLOC], tt0[:], STEP, p[:, 0:BLOC],
                    op0=ALU.mult, op1=ALU.add)
                nc.vector.scalar_tensor_tensor(
                    m[:, BLOC:2 * BLOC], tt1[:], STEP, p[:, BLOC:2 * BLOC],
                    op0=ALU.mult, op1=ALU.add)

                # output row t: transpose state copy [128, 2b] -> [b, 256]
                # into a GRP-step PSUM batch; emitted one iteration late so the
                # next step's V matmuls outrank the transposes when xq lands
                pend.append((t, g, gn, xq0, xq1))
                if len(pend) == 2 or t == n_steps:
                    for (tp, gp, gnp, xqp0, xqp1) in (pend if t == n_steps else pend[:1]):
                        if gp == 0:
                            pt = pst.tile([BLOC, GRP, N], F16, tag="pt", name="pt")
                        nc.tensor.transpose(pt[:, gp, 0:128], xqp0[:], ident[:])
                        nc.tensor.transpose(pt[:, gp, 128:256], xqp1[:], ident[:])
                        if gp == gnp - 1:
                            pend2.append((tp, gnp, pt))
                    pend = [] if t == n_steps else pend[1:]
                # stage copies one extra iteration late, so their sem waits are
                # already satisfied when they enter the DVE/ACT queues
                if t == n_steps:
                    ready, pend2 = pend2, []
                else:
                    ready = [e for e in pend2 if e[0] <= t - 2]
                    pend2 = [e for e in pend2 if e[0] > t - 2]
                for (tp, gnp, ptp) in ready:
                    stage = stpool.tile([BLOC, GRP, N], F32, tag="stage", name="stage")
                    half = (gnp + 1) // 2
                    nc.vector.tensor_copy(stage[:, 0:half], ptp[:, 0:half])
                    nc.scalar.copy(stage[:, half:gnp], ptp[:, half:gnp])
                    nc.sync.dma_start(OUT[:, tp - gnp + 1:tp + 1, :], stage[:, 0:gnp])
    nc.compile()
    return nc


def kernel(X0, MA, MW, bA_z, bW_z, by_w):
    global LAST_RESULT
    from concourse.bass_utils import run_bass_kernel_spmd

    X0 = np.asarray(X0, dtype=np.float32)
    MA = np.asarray(MA, dtype=np.float32)
    MW = np.asarray(MW, dtype=np.float32)
    bA_z = np.asarray(bA_z, dtype=np.float32)
    bW_z = np.asarray(bW_z, dtype=np.float32)
    by_w = np.asarray(by_w, dtype=np.float32)

    # host-side weight prep (f32, matches reference math); weights to fp16
    # for full-rate PE matmuls (master state stays fp32 on device).
    bA = np.float32(0.5) * np.exp(-bA_z[0, 0] * bA_z[0, 0]) + np.float32(0.5)
    bW = np.float32(0.5) * np.exp(-bW_z[0, 0] * bW_z[0, 0]) + np.float32(0.5)
    I = np.eye(N, dtype=np.float32)
    A = (1 - bA) * (MA + MA.T) + bA * (MA - MA.T) - np.float32(YA) * I
    C = (1 - bA) * (MW + MW.T) + bW * (MW - MW.T) - np.float32(YW) * I
    WA = np.ascontiguousarray(A.T).astype(np.float16)
    WC = np.ascontiguousarray(C.T).astype(np.float16)

    n_steps = TMAX - 1
    in_maps = []
    for i in range(NCORES):
        in_maps.append({
            "WA": WA,
            "WC": WC,
            "BY": by_w,
            "X0T": np.ascontiguousarray(X0[i * BLOC:(i + 1) * BLOC, :].T),
        })

    nc = _build(n_steps)
    res = run_bass_kernel_spmd(nc, in_maps, core_ids=list(range(NCORES)))
    LAST_RESULT = res

    out = np.concatenate([r["OUT"] for r in res.results], axis=0)
    out[:, 0, :] = X0
    return out


if __name__ == "__main__":
    rng = np.random.default_rng(0)
    inputs = {
        "X0": rng.standard_normal((BS, N), dtype=np.float32),
        "MA": rng.standard_normal((N, N), dtype=np.float32) / 16,
        "MW": rng.standard_normal((N, N), dtype=np.float32) / 16,
        "bA_z": np.full((1, 1), 0.65, dtype=np.float32),
        "bW_z": np.full((1, 1), 0.65, dtype=np.float32),
        "by_w": rng.standard_normal((N, 1), dtype=np.float32) / 100,
    }
    out = kernel(**inputs)
    print("out", out.shape, out.dtype, np.abs(out).max())
